# revision 56
# baseline (speedup 1.0000x reference)
"""Trainium2 Bass kernel for nn_LossFunction_49615462203607 (GlobalAlignLoss +
keypoint/knn consensus losses).

Reference computation (B=4, N=4096, KP=512, K=32):
  P[b,i,j] = ||tgt[b,:,i] - src_transformed[b,:,j]||^2   (squared euclidean)
  loss_1 = sum_{b,j} huber(min_i P[b,i,j], c)   (colmins, per src point)
  loss_2 = sum_{b,i} huber(min_j P[b,i,j], c)   (rowmins, per tgt point)
  gal    = loss_1 + loss_2,  c = 0.01
  kp_loss  = sum (R@src_kp + t - tgt_kp)^2
  knn_loss = sum (src_knn - tgt_knn)^2
  ncl      = knn_loss/k + kp_loss
  returns (ncl, gal)

Strategy (8 cores = 4 batches x 2 row-halves, single-matrix scheme):
  - Each core computes its 2048x4096 slab of P ONCE via augmented K=5 fp16
    matmuls: host-built operands xa = [1, xx, x] (x = its tgt rows) and
    ya = [yy, 1, -2y] (y = full src cloud), so lhsT.T @ rhs = xx+yy-2x.y.
  - PE throughput here is 1 free-col/cycle @1.2GHz (427ns per 512-col MM,
    measured; the PE never ramps past mid p-state on this part), so the
    two-pass formulation (both min directions as free-dim reductions) would
    be PE-bound at ~109us/core. Computing the matrix once halves PE time.
  - ScalarE (ACT) stages every PSUM tile to SBUF fp16 (1 elem/lane/cyc,
    ~2us per [128,2048] round) - ACT paces the steady-state rounds.
  - VectorE consumes the fp16 staged data, where 16-bit TT min runs at
    2x_1P (2 out/cycle = 4 fresh elems/cycle on fresh-pair folds):
      rowmins: binary fold tree batched over 8 i-tiles via 3D APs (the
        ~141cyc/instr fixed cost amortizes), finished by a 3D tensor_reduce.
      colmins: running elementwise TT min into a [128, 2048] accumulator
        per j-half (partial over this core's 2048 rows).
    VectorE is the critical engine (~99us busy).
  - Column-min partials are partition-reduced via 16 PE transposes
    ([128,128] fp16 -> PSUM) + one tensor_reduce, and shipped out raw
    ([128, 32] per core). Sweep order is r (j-half) outer, i-tile inner;
    the r=0 finale + late fold residues are deferred into the r=1 sweep
    (r=1's matmuls only need r=0's last stage to free its PSUM tile, so
    the transition never stalls on the DVE backlog).
  - Host gather: sums scalar partials; for colmins, elementwise-min of the
    two row-half partials per batch, then huber+sum (4 x 4096 elements).
  - huber for rowmins + keypoint/knn losses on-chip as before.

All reduce-type DVE ops measured 1x regardless of dtype; only plain
TENSOR_TENSOR reaches 2x (16-bit, SBUF) - hence the fold-tree design.
GpSimd compute ops (tensor_tensor/tensor_scalar) fail walrus ISA checks on
the Pool engine in this toolchain, so it only runs memsets/iota/DMAs here.
"""

from contextlib import ExitStack

import numpy as np

import concourse.bacc as bacc
import concourse.mybir as mybir
import concourse.tile as tile
from concourse import bass_utils
from concourse import masks


F32 = mybir.dt.float32
FP16 = mybir.dt.float16
ALU = mybir.AluOpType

B = 4
N = 4096          # points per cloud
KP = 512          # keypoints
MARGIN = 0.01
NCORES = 8
HALF = N // 2     # i-rows per core
JT = 512          # free dim per matmul (one PSUM bank of fp32)
KAUG = 5          # augmented contraction rows: [1, xx, x] x [yy, 1, -2y]
IT = 128          # i-tile partition count
N_ITILES = HALF // IT        # 16
RND_J = N // 2               # j-columns per round tile (4 banks)
GRP = 8                      # i-tiles per rowmin fold group
KP_H = KP // 2               # keypoints per core
INF16 = 30000.0              # > max possible squared distance (~300)

_BUILD_CACHE: dict = {}


def _stage_aug(nc, pools, xa_dram, ya_dram):
    """DMA the host-built augmented fp16 operands straight into SBUF.
    xa rows [1, xx, x0, x1, x2]; ya rows [yy, 1, -2y0, -2y1, -2y2] so
    lhsT.T @ rhs = xx_i + yy_j - 2 x_i.y_j with a K=5 contraction. fp16
    streams 1 row/cycle on the PE like f32r but without the compute-written
    producer requirement, so plain DMAs suffice; the ya second half is
    deferred (only the r=1 sweep needs it)."""
    aug = pools["aug"]
    xa = aug.tile([KAUG, HALF], FP16, name="xa")
    ya = aug.tile([KAUG, N], FP16, name="ya")
    nc.sync.dma_start(out=ya[:, 0:N // 2], in_=ya_dram[:, 0:N // 2])
    nc.sync.dma_start(out=xa, in_=xa_dram[:, :])
    return xa, ya


def _build(kinv: float):
    nc = bacc.Bacc("TRN2", target_bir_lowering=False, debug=False)

    # ---- DRAM I/O (per-core shards supplied by host) ----
    pa_x = nc.dram_tensor("pa_x", [KAUG, HALF], FP16, kind="ExternalInput").ap()
    pa_y = nc.dram_tensor("pa_y", [KAUG, N], FP16, kind="ExternalInput").ap()
    # kp_src4 = [ones ; src_kp] so [t ; R^T].T @ [1 ; src] = R@src + t
    kp_src4 = nc.dram_tensor("kp_src4", [4, KP_H], F32, kind="ExternalInput").ap()
    kp_tgt = nc.dram_tensor("kp_tgt", [3, KP_H], F32, kind="ExternalInput").ap()
    rt4 = nc.dram_tensor("rt4", [4, 3], F32, kind="ExternalInput").ap()
    # knn_both = [src_knn | tgt_knn] as one [96, 2*KP_H] tensor
    knn_both = nc.dram_tensor("knn_both", [96, 2 * KP_H], F32, kind="ExternalInput").ap()
    part = nc.dram_tensor("part", [1, 2], F32, kind="ExternalOutput").ap()
    colp_d = nc.dram_tensor("colp", [IT, 2 * N_ITILES], F32, kind="ExternalOutput").ap()

    with ExitStack() as ctx:
        tc = ctx.enter_context(tile.TileContext(nc))
        pools = {
            "aug": ctx.enter_context(tc.tile_pool(name="aug", bufs=1)),
            "psum": ctx.enter_context(tc.tile_pool(name="psum", bufs=2, space="PSUM")),
            "stg": ctx.enter_context(tc.tile_pool(name="stg", bufs=3)),
            "fold": ctx.enter_context(tc.tile_pool(name="fold", bufs=1)),
            "small": ctx.enter_context(tc.tile_pool(name="small", bufs=2)),
            "consts": ctx.enter_context(tc.tile_pool(name="consts", bufs=1)),
        }
        consts = pools["consts"]
        small = pools["small"]
        psum = pools["psum"]
        stgp = pools["stg"]
        fold = pools["fold"]

        ones = consts.tile([128, 1], F32)
        nc.vector.memset(ones, 1.0)
        # colmin accumulators (one per j-half) + rowmin partials (per sweep)
        colacc = [consts.tile([IT, RND_J], FP16, name=f"colacc{r}") for r in range(2)]
        for r in range(2):
            nc.vector.memset(colacc[r], INF16)
        rowmins = [consts.tile([IT, N_ITILES], F32, name=f"rowm{r}") for r in range(2)]
        colp = consts.tile([IT, 2 * N_ITILES], F32, name="colp_sb")

        xa, ya = _stage_aug(nc, pools, pa_x, pa_y)

        # small-loss inputs via the gpsimd software-DGE queue so they don't
        # serialize behind the aug staging on the sync HWDGE queue
        kps4 = small.tile([4, KP_H], F32, tag="kp", name="kps4")
        kpt = small.tile([3, KP_H], F32, tag="kp", name="kpt")
        rt = small.tile([4, 3], F32, tag="rt")
        knb = small.tile([96, 2 * KP_H], F32, tag="knn", name="knb")
        nc.gpsimd.dma_start(out=kps4, in_=kp_src4[:, :])
        nc.gpsimd.dma_start(out=kpt, in_=kp_tgt[:, :])
        nc.gpsimd.dma_start(out=rt, in_=rt4[:, :])
        nc.gpsimd.dma_start(out=knb, in_=knn_both[:, :])
        ident = consts.tile([IT, IT], FP16, name="ident")
        masks.make_identity(nc, ident)

        # second ya half, deferred: only the r=1 sweep needs it
        nc.sync.dma_start(out=ya[:, N // 2:N], in_=pa_y[:, N // 2:N])

        def colmin_group(stg, r, gsz):
            """Accumulate colacc[r] with a pair-tree over the group's staged
            tiles (fresh-pair TT folds run at 4 fresh elems/cycle)."""
            if gsz == 8:
                u4 = fold.tile([IT, 4, RND_J], FP16, tag="u4", name=f"u4_{r}")
                nc.vector.tensor_tensor(out=u4, in0=stg[:, 0:8:2, :],
                                        in1=stg[:, 1:8:2, :], op=ALU.min)
                u2 = fold.tile([IT, 2, RND_J], FP16, tag="u2", name=f"u2_{r}")
                nc.vector.tensor_tensor(out=u2, in0=u4[:, 0:4:2, :],
                                        in1=u4[:, 1:4:2, :], op=ALU.min)
                nc.vector.tensor_tensor(out=u2[:, 0, :], in0=u2[:, 0, :],
                                        in1=u2[:, 1, :], op=ALU.min)
                nc.vector.tensor_tensor(out=colacc[r], in0=u2[:, 0, :],
                                        in1=colacc[r], op=ALU.min)
            elif gsz == 6:
                u3 = fold.tile([IT, 3, RND_J], FP16, tag="u3", name=f"u3_{r}")
                nc.vector.tensor_tensor(out=u3, in0=stg[:, 0:6:2, :],
                                        in1=stg[:, 1:6:2, :], op=ALU.min)
                nc.vector.tensor_tensor(out=u3[:, 0, :], in0=u3[:, 0, :],
                                        in1=u3[:, 1, :], op=ALU.min)
                nc.vector.tensor_tensor(out=u3[:, 0, :], in0=u3[:, 0, :],
                                        in1=u3[:, 2, :], op=ALU.min)
                nc.vector.tensor_tensor(out=colacc[r], in0=u3[:, 0, :],
                                        in1=colacc[r], op=ALU.min)
            else:
                for ti in range(gsz):
                    nc.vector.tensor_tensor(out=colacc[r], in0=stg[:, ti, :],
                                            in1=colacc[r], op=ALU.min)

        def fold_l1(stg, m1, a, b):
            """First fold level for staged tiles [a:b) of a group."""
            nc.vector.tensor_tensor(out=m1[:, a:b, :], in0=stg[:, a:b, 0:1024],
                                    in1=stg[:, a:b, 1024:2048], op=ALU.min)

        def fold_rest(m1, r, t0, gsz):
            """Fold levels 2..5 + reduce -> rowmins[r][:, t0:t0+gsz]."""
            m2 = fold.tile([IT, GRP, 512], FP16, tag="m2", name=f"m2_{r}_{t0}")
            nc.vector.tensor_tensor(out=m2[:, 0:gsz, :], in0=m1[:, 0:gsz, 0:512],
                                    in1=m1[:, 0:gsz, 512:1024], op=ALU.min)
            m3 = fold.tile([IT, GRP, 256], FP16, tag="m3", name=f"m3_{r}_{t0}")
            nc.vector.tensor_tensor(out=m3[:, 0:gsz, :], in0=m2[:, 0:gsz, 0:256],
                                    in1=m2[:, 0:gsz, 256:512], op=ALU.min)
            m4 = fold.tile([IT, GRP, 128], FP16, tag="m4", name=f"m4_{r}_{t0}")
            nc.vector.tensor_tensor(out=m4[:, 0:gsz, :], in0=m3[:, 0:gsz, 0:128],
                                    in1=m3[:, 0:gsz, 128:256], op=ALU.min)
            m5 = fold.tile([IT, GRP, 64], FP16, tag="m5", name=f"m5_{r}_{t0}")
            nc.vector.tensor_tensor(out=m5[:, 0:gsz, :], in0=m4[:, 0:gsz, 0:64],
                                    in1=m4[:, 0:gsz, 64:128], op=ALU.min)
            nc.vector.tensor_reduce(out=rowmins[r][:, t0:t0 + gsz],
                                    in_=m5[:, 0:gsz, :],
                                    axis=mybir.AxisListType.X, op=ALU.min)

        def finale(r):
            """Partition-reduce colacc[r] -> colp[:, r*16:(r+1)*16]."""
            tp = psum.tile([IT, RND_J], F32, tag="ps", name=f"tp{r}")
            tph = tp.bitcast(FP16)
            for c in range(N_ITILES):
                nc.tensor.transpose(tph[:, c * IT:(c + 1) * IT],
                                    colacc[r][:, c * IT:(c + 1) * IT], ident)
            nc.vector.tensor_reduce(
                out=colp[:, r * N_ITILES:(r + 1) * N_ITILES],
                in_=tph.rearrange("p (c f) -> p c f", f=IT)[:, 0:N_ITILES, :],
                axis=mybir.AxisListType.X, op=ALU.min)

        # ---- keypoint + knn losses (emitted after the first main round
        # so their gpsimd-DMA deps don't gate the first matmuls/stages) ----
        kp_col = consts.tile([3, 1], F32)
        knn_col = consts.tile([96, 1], F32)

        def emit_kp_losses():
            kp_t = psum.tile([IT, RND_J], F32, tag="ps", name="kp_t")
            kp_ps = kp_t[0:3, 0:KP_H]
            nc.tensor.matmul(out=kp_ps, lhsT=rt, rhs=kps4, start=True, stop=True)
            kp_d = small.tile([3, KP_H], F32, tag="kp", name="kp_d")
            nc.vector.tensor_sub(kp_d, kp_ps, kpt)
            kp_sq = small.tile([3, KP_H], F32, tag="kp", name="kp_sq")
            nc.scalar.square(kp_sq, kp_d)
            nc.vector.reduce_sum(kp_col, kp_sq, axis=mybir.AxisListType.X)
            kd = small.tile([96, KP_H], F32, tag="knnd", name="kd")
            nc.vector.tensor_sub(kd, knb[:, 0:KP_H], knb[:, KP_H:])
            kd_sq = small.tile([96, KP_H], F32, tag="knnd", name="kd_sq")
            nc.scalar.square(kd_sq, kd)
            nc.vector.reduce_sum(knn_col, kd_sq, axis=mybir.AxisListType.X)
            nc.scalar.mul(knn_col, knn_col, kinv)

        # ---- main loop: j-half sweeps (r outer so finale overlaps) ----
        # Group sizes taper toward the sweep end, L1 folds are emitted in
        # half-group chunks to smooth DVE load, and the later groups' fold
        # residuals are deferred until AFTER the finale TR so the last
        # colmins (and with them the finale transposes + the PSUM buffer
        # rotation) aren't stuck behind multi-us folds in the DVE queue.
        GROUPS = [(0, 8), (8, 8)]
        pending = []
        for r in range(2):
            for gi, (t0, gsz) in enumerate(GROUPS):
                stg = stgp.tile([IT, GRP, RND_J], FP16, tag="stg",
                                name=f"stg{r}_{t0}")
                m1 = fold.tile([IT, GRP, 1024], FP16, tag="m1", bufs=2,
                               name=f"m1_{r}_{t0}")
                for ti in range(gsz):
                    t = t0 + ti
                    ps = psum.tile([IT, RND_J], F32, tag="ps", name=f"ps{r}_{t}")
                    lhsT = xa[:, t * IT:(t + 1) * IT]
                    for bk in range(4):
                        j0 = r * RND_J + bk * JT
                        nc.tensor.matmul(out=ps[:, bk * JT:(bk + 1) * JT],
                                         lhsT=lhsT, rhs=ya[:, j0:j0 + JT],
                                         start=True, stop=True)
                    nc.scalar.copy(stg[:, ti, :], ps[:, :])
                    nc.vector.tensor_tensor(out=colacc[r], in0=stg[:, ti, :],
                                            in1=colacc[r], op=ALU.min)
                    if (r, gi, ti) == (1, 0, 2) and pending:
                        # r0's finale + deferred fold residues flush here:
                        # they overlap the r1 sweep instead of stalling the
                        # transition (r1's matmuls only need r0's last
                        # stage to free its PSUM tile).
                        for fn in pending:
                            fn()
                        pending = []
                    if (r, gi, ti) == (0, 0, 1):
                        emit_kp_losses()
                    if (r, gi) == (0, 0) and ti % 2 == 1:
                        # finer L1 chunks fill the DVE pipeline-fill idle
                        # at the very start of the kernel
                        fold_l1(stg, m1, ti - 1, ti + 1)
                    elif gi == 0 and ti % 2 == 1:
                        fold_l1(stg, m1, ti - 1, ti + 1)
                    elif gi == 1 and ti == gsz // 2 - 1:
                        fold_l1(stg, m1, 0, gsz // 2)

                if gi == 0:
                    fold_rest(m1, r, t0, gsz)
                elif gi == 1:
                    pending.append(lambda stg=stg, m1=m1, r=r, t0=t0, gsz=gsz: (
                        fold_l1(stg, m1, gsz // 2, gsz),
                        fold_rest(m1, r, t0, gsz)))
                else:
                    pending.append(lambda stg=stg, m1=m1, r=r, t0=t0, gsz=gsz: (
                        fold_l1(stg, m1, 0, gsz), fold_rest(m1, r, t0, gsz)))
            pending.insert(0, lambda r=r: finale(r))
        for fn in pending:
            fn()
        # colp is complete once finale(1)'s reduce lands; ship it while the
        # tail folds/huber still run
        nc.sync.dma_start(out=colp_d[:, :], in_=colp)

        # ---- rowmin huber + reduce ----
        rowmin = small.tile([IT, N_ITILES], F32, tag="hub", bufs=4, name="rowmin")
        nc.vector.tensor_tensor(out=rowmin, in0=rowmins[0], in1=rowmins[1],
                                op=ALU.min)
        # huber(x) = 0.5*min(x,c)^2 + c*relu(x-c)
        hub_m = small.tile([IT, N_ITILES], F32, tag="hub", bufs=4, name="hub_m")
        nc.vector.tensor_scalar(
            out=hub_m, in0=rowmin, scalar1=MARGIN, scalar2=float(np.sqrt(0.5)),
            op0=ALU.min, op1=ALU.mult,
        )
        # all-DVE huber tail: avoids DVE<->ACT semaphore hops on the
        # serial end-of-kernel path
        hub_m2 = small.tile([IT, N_ITILES], F32, tag="hub", bufs=4, name="hub_m2")
        nc.vector.tensor_mul(hub_m2, hub_m, hub_m)
        hub_r = small.tile([IT, N_ITILES], F32, tag="hub", bufs=4, name="hub_r")
        nc.vector.tensor_scalar(
            out=hub_r, in0=rowmin, scalar1=MARGIN, scalar2=0.0,
            op0=ALU.subtract, op1=ALU.max,
        )
        hub_rc = small.tile([IT, N_ITILES], F32, tag="hub2", bufs=2, name="hub_rc")
        nc.vector.tensor_scalar_mul(hub_rc, hub_r, MARGIN)
        hub_full = small.tile([IT, N_ITILES], F32, tag="hub2", bufs=2, name="hub_full")
        nc.vector.tensor_add(hub_full, hub_m2, hub_rc)
        gal_col = consts.tile([IT, 1], F32)
        nc.vector.reduce_sum(gal_col, hub_full, axis=mybir.AxisListType.X)

        # ---- cross-partition sums via matmul with ones ----
        sum_t = psum.tile([IT, RND_J], F32, tag="ps", name="sum_t")
        ncl_ps = sum_t[0:1, 0:1]
        nc.tensor.matmul(out=ncl_ps, lhsT=knn_col, rhs=ones[0:96, :],
                         start=True, stop=False)
        nc.tensor.matmul(out=ncl_ps, lhsT=kp_col, rhs=ones[0:3, :],
                         start=False, stop=True)
        gal_ps = sum_t[0:1, 1:2]
        nc.tensor.matmul(out=gal_ps, lhsT=gal_col, rhs=ones[:, :],
                         start=True, stop=True)

        outsb = consts.tile([1, 2], F32)
        nc.scalar.copy(outsb[:, 0:1], ncl_ps)
        nc.scalar.copy(outsb[:, 1:2], gal_ps[0:1, 0:1])
        nc.sync.dma_start(out=part[:, :], in_=outsb)
        nc.sync.dma_start(out=colp_d[:, :], in_=colp)

    nc.compile()
    return nc


def _get_nc(kinv: float):
    key = round(kinv, 12)
    if key not in _BUILD_CACHE:
        _BUILD_CACHE[key] = _build(kinv)
    return _BUILD_CACHE[key]


def make_in_maps(src_keypoints, tgt_keypoints, rotation_ab, translation_ab,
                 src_keypoints_knn, tgt_keypoints_knn, src_transformed, tgt):
    """Shard full inputs into the 8 per-core input maps (layout/slicing only)."""
    a = lambda x: np.ascontiguousarray(np.asarray(x, dtype=np.float32))
    ones_row = np.ones((1, KP_H), dtype=np.float32)
    in_maps = []
    for c in range(NCORES):
        b, h = c // 2, c % 2
        sl = slice(h * HALF, (h + 1) * HALF)
        kpsl = slice(h * KP_H, (h + 1) * KP_H)
        knn_s = (
            np.asarray(src_keypoints_knn)[b][:, kpsl, :]
            .transpose(0, 2, 1)
            .reshape(96, KP_H)
        )
        knn_t = (
            np.asarray(tgt_keypoints_knn)[b][:, kpsl, :]
            .transpose(0, 2, 1)
            .reshape(96, KP_H)
        )
        x = np.asarray(tgt, dtype=np.float64)[b][:, sl]           # [3, HALF]
        y = np.asarray(src_transformed, dtype=np.float64)[b]      # [3, N]
        xa = np.concatenate([
            np.ones((1, HALF)), (x * x).sum(axis=0, keepdims=True), x,
        ], axis=0)
        ya = np.concatenate([
            (y * y).sum(axis=0, keepdims=True), np.ones((1, N)), -2.0 * y,
        ], axis=0)
        in_maps.append({
            "pa_x": np.ascontiguousarray(xa.astype(np.float16)),
            "pa_y": np.ascontiguousarray(ya.astype(np.float16)),
            "kp_src4": a(np.concatenate(
                [ones_row, np.asarray(src_keypoints)[b][:, kpsl]], axis=0)),
            "kp_tgt": a(tgt_keypoints[b][:, kpsl]),
            "rt4": a(np.concatenate([
                np.asarray(translation_ab)[b][None, :],
                np.asarray(rotation_ab)[b].T,
            ], axis=0)),
            "knn_both": a(np.concatenate([knn_s, knn_t], axis=1)),
        })
    return in_maps


_RUNNER_CACHE: dict = {}


def _get_runner(kinv: float):
    """Build the 8-core jitted executable once and reuse it across calls
    (run_bass_via_pjrt re-traces the jit on every invocation)."""
    key = round(kinv, 12)
    if key in _RUNNER_CACHE:
        return _RUNNER_CACHE[key]

    import jax
    from jax.experimental.shard_map import shard_map
    from jax.sharding import Mesh, PartitionSpec
    import concourse.bass2jax as bass2jax
    import concourse.mybir as _mb

    nc = _get_nc(kinv)
    bass2jax.install_neuronx_cc_hook()

    part_name = nc.partition_id_tensor.name if nc.partition_id_tensor else None
    in_names, out_names, out_avals = [], [], []
    for alloc in nc.m.functions[0].allocations:
        if not isinstance(_mb.MemoryLocationSet, type) or not isinstance(
            alloc, _mb.MemoryLocationSet
        ):
            continue
        name = alloc.memorylocations[0].name
        if alloc.kind == "ExternalInput":
            if name != part_name:
                in_names.append(name)
        elif alloc.kind == "ExternalOutput":
            out_names.append(name)
            out_avals.append(
                jax.core.ShapedArray(
                    tuple(alloc.tensor_shape), _mb.dt.np(alloc.dtype)
                )
            )
    n_params = len(in_names)
    all_in_names = in_names + out_names
    if part_name is not None:
        all_in_names = all_in_names + [part_name]

    def _body(*args):
        operands = list(args)
        if part_name is not None:
            operands.append(bass2jax.partition_id_tensor())
        outs = bass2jax._bass_exec_p.bind(
            *operands,
            out_avals=tuple(out_avals),
            in_names=tuple(all_in_names),
            out_names=tuple(out_names),
            lowering_input_output_aliases=(),
            sim_require_finite=True,
            sim_require_nnan=True,
            nc=nc,
        )
        return tuple(outs)

    devices = jax.devices()[:NCORES]
    mesh = Mesh(np.asarray(devices), ("core",))
    n_outs = len(out_names)
    sharded = jax.jit(
        shard_map(
            _body,
            mesh=mesh,
            in_specs=(PartitionSpec("core"),) * (n_params + n_outs),
            out_specs=(PartitionSpec("core"),) * n_outs,
            check_rep=False,
        ),
        donate_argnums=tuple(range(n_params, n_params + n_outs)),
        keep_unused=True,
    )

    def run(in_maps):
        concat_in = [
            np.concatenate([m[name] for m in in_maps], axis=0) for name in in_names
        ]
        concat_zeros = [
            np.zeros((NCORES * a.shape[0], *a.shape[1:]), a.dtype) for a in out_avals
        ]
        out_arrs = sharded(*concat_in, *concat_zeros)
        return [
            {
                name: np.asarray(out_arrs[i]).reshape(
                    NCORES, *out_avals[i].shape
                )[c]
                for i, name in enumerate(out_names)
            }
            for c in range(NCORES)
        ]

    _RUNNER_CACHE[key] = run
    return run


def _huber64(x):
    c = MARGIN
    return np.where(x < c, 0.5 * x * x, c * x - 0.5 * c * c)


def kernel(src_keypoints, tgt_keypoints, rotation_ab, translation_ab,
           src_keypoints_knn, tgt_keypoints_knn, k, src_transformed, tgt,
           _trace=False):
    k_val = float(np.asarray(k))
    in_maps = make_in_maps(
        src_keypoints, tgt_keypoints, rotation_ab, translation_ab,
        src_keypoints_knn, tgt_keypoints_knn, src_transformed, tgt,
    )
    if _trace:
        nc = _get_nc(1.0 / k_val)
        res = bass_utils.run_bass_kernel_spmd(
            nc, in_maps, core_ids=list(range(NCORES)), trace=True
        )
        results = res.results
    else:
        run = _get_runner(1.0 / k_val)
        results = run(in_maps)
        res = None
    parts = np.stack([r["part"] for r in results])   # [8, 1, 2]
    colps = np.stack([r["colp"] for r in results]).astype(np.float64)  # [8,128,32]
    ncl = parts[:, 0, 0].astype(np.float64).sum()
    gal = parts[:, 0, 1].astype(np.float64).sum()
    # colmin combine: per batch, elementwise min of the two row-half partials
    for b in range(B):
        m = np.minimum(colps[2 * b], colps[2 * b + 1])
        gal += _huber64(m).sum()
    out = (np.float32(ncl), np.float32(gal))
    if _trace:
        return out, res
    return out
